# revision 19
# baseline (speedup 1.0000x reference)
"""ComputeAlignmentError kernel for 8 TRN2 NeuronCores.

Math: for each batch, pairwise alignment error
    err[i,j] = || Ep_j (pc_i - bp_j) - Et_j (tc_i - bt_j) + eps ||_2
where Ep/Et are orthonormal frame bases built from pred/true frames and
bp/bt the frame origins.  The eps terms contribute O(1e-8) relative and
are dropped; since Ep/Et are rotations the error collapses to a rank-17
bilinear form  err^2[i,j] = Y[i] . Z[j]:
    Y[i] = [1, |pc|^2+|tc|^2, pc, tc, vec(pc tc^T)]          (17)
    Z[j] = [z0, 1, -2bp - S bt, -2bt - S^T bp, vec(S)]       (17)
    S_j  = -2 Ep_j^T Et_j,   z0 = bp.(S bt + bp) + |bt|^2
Mask folds in for free: Y *= mask_i, Z *= mask_j.

Each core handles one (batch, 512-row i-slice).  Output is computed
j-major.  The j range is processed in TWO half-pipelines of 8 chunks
each so the scalar-engine sqrt drain of half A overlaps the vector
feature chain of half B: per half, Z features for 8x128 j are built
on-chip ([128 j, 8 chunks, 32 feat]), PE-transposed in groups of 4
chunks (chunk c lands at PE row group 32*(c%4) -- no replication for
Z), then one f32r matmul [17,128]x[17,512] per chunk into PSUM pairs,
sqrt over [128,1024] (scalar ACT, fused +bias guard against f32r
rounding pushing err^2<0), and a 512KB DMA per pair into a j-major
[2048, 512] output the host transposes back.  Y ([17, 512]) is built
once on scalar/gpsimd, PE-transposed, and replicated to all 4 row
groups with engine copies.
"""

import os
import sys

import numpy as np

sys.path.insert(0, "/opt/trn_rl_repo")

from contextlib import ExitStack

import concourse.bacc as bacc
import concourse.bass as bass
import concourse.tile as tile
from concourse import mybir
from concourse.bass_utils import run_bass_kernel_spmd
from concourse.masks import make_identity

F32 = mybir.dt.float32
AF = mybir.ActivationFunctionType

B, N = 2, 2048
NCORES = 8
ISLICE = N * B // NCORES  # 512 rows of i per core
NITILE = ISLICE // 128  # 4 i-chunks per core
NJCH = N // 128  # 16 j-chunks
NF = 17  # feature count K
FPAD = 32  # feature slot padding (PE row-group / PSUM alignment)
HALF = NJCH // 2  # chunks per half-pipeline

USE_F32R = True  # single-pass PE matmul; guarded by SQRT_BIAS
SQRT_BIAS = 2e-2 if USE_F32R else 2e-4


def _build(nc_holder=[]):
    if nc_holder:
        return nc_holder[0]
    nc = bacc.Bacc(
        "TRN2",
        target_bir_lowering=False,
        debug=False,
        enable_asserts=True,
        num_devices=NCORES,
    )
    # frames: [128, chunk, set, pt, xyz] (chunk-major so each half is
    # contiguous); coords: [128, chunk, set, xyz]
    frames_in = nc.dram_tensor("frames", [128, NJCH * 2 * 9], F32, kind="ExternalInput").ap()
    coords_in = nc.dram_tensor("coords", [128, NITILE * 6], F32, kind="ExternalInput").ap()
    maskj_in = nc.dram_tensor("maskj", [128, NJCH], F32, kind="ExternalInput").ap()
    maski_in = nc.dram_tensor("maski", [128, NITILE], F32, kind="ExternalInput").ap()
    out_dram = nc.dram_tensor("out", [N, ISLICE], F32, kind="ExternalOutput").ap()

    with tile.TileContext(nc) as tc, ExitStack() as ctx:
        _kernel_body(ctx, tc, out_dram, frames_in, coords_in, maskj_in, maski_in)

    nc.compile()
    nc_holder.append(nc)
    return nc


def _half_chain(nc, sb, Zb, Ft, Mj, h):
    """Emit the Z-feature chain for half h (chunks h*HALF .. h*HALF+7).

    Ft is the [P, NJCH, 2(set), 3(pt), 3(xyz)] frames tile; Zb is this
    half's [P, HALF, FPAD] feature buffer.  Vector carries the critical
    chain; scalar does squares/sqrts/copies that feed it.
    """
    P = 128
    G = 2 * HALF  # 16 groups: (chunk, set) chunk-major
    t = f"h{h}"
    Fh = Ft[:, h * HALF : (h + 1) * HALF]  # [P, 8, 2, 3, 3]
    Fg = Fh.rearrange("p c s t x -> p (c s) t x")  # [P, G, 3, 3]
    bp = Fh[:, :, 0, 1, :]  # [P, 8, 3]
    bt = Fh[:, :, 1, 1, :]

    # bpt_m2 = -2 * origins (gpsimd, off-chain)
    bpt_m2 = sb.tile([P, HALF, 2, 3], F32, tag=f"bptm2{t}")
    nc.gpsimd.tensor_scalar_mul(bpt_m2[:], Fh[:, :, :, 1, :], -2.0)
    # |bt|^2 terms for z0 (scalar, off-chain)
    m2 = sb.tile([P, HALF, 2, 3], F32, tag=f"m2{t}")
    nc.scalar.square(m2[:, :, 1, :], bt)

    w12 = sb.tile([P, G, 2, 3], F32, tag=f"w12{t}")
    nc.vector.tensor_sub(
        w12[:],
        Fg[:, :, 0::2, :],
        Fg[:, :, 1, :].unsqueeze(2).broadcast_to((P, G, 2, 3)),
    )
    pr = sb.tile([P, G, 3, 3], F32, tag=f"pr{t}")
    nc.scalar.square(pr[:, :, 0:2, :], w12[:])
    nc.vector.tensor_mul(pr[:, :, 2, :], w12[:, :, 0, :], w12[:, :, 1, :])
    dots = sb.tile([P, G, 3], F32, tag=f"dots{t}")
    nc.vector.reduce_sum(dots[:].unsqueeze(3), pr[:], axis=mybir.AxisListType.X)
    nrm12 = sb.tile([P, G, 2], F32, tag=f"nrm12{t}")
    nc.scalar.sqrt(nrm12[:], dots[:, :, 0:2])
    rinv12 = sb.tile([P, G, 2], F32, tag=f"rinv12{t}")
    nc.vector.reciprocal_approx_fast(
        rinv12[:].rearrange("p g w -> p (g w)"), nrm12[:].rearrange("p g w -> p (g w)")
    )
    w12n = sb.tile([P, G, 2, 3], F32, tag=f"w12n{t}")
    nc.vector.tensor_mul(w12n[:], w12[:], rinv12[:].unsqueeze(3).broadcast_to((P, G, 2, 3)))
    e12p = sb.tile([P, G, 2, 3], F32, tag=f"e12p{t}")
    nc.vector.tensor_add(e12p[:, :, 0, :], w12n[:, :, 0, :], w12n[:, :, 1, :])
    nc.vector.tensor_sub(e12p[:, :, 1, :], w12n[:, :, 1, :], w12n[:, :, 0, :])
    sq2 = sb.tile([P, G, 2, 3], F32, tag=f"sq2{t}")
    nc.scalar.square(sq2[:], e12p[:])
    n2b = sb.tile([P, G, 2], F32, tag=f"n2b{t}")
    nc.vector.reduce_sum(n2b[:].unsqueeze(3), sq2[:], axis=mybir.AxisListType.X)
    nrmb = sb.tile([P, G, 2], F32, tag=f"nrmb{t}")
    nc.scalar.sqrt(nrmb[:], n2b[:])
    uv = sb.tile([P, G, 2], F32, tag=f"uv{t}")
    nc.vector.reciprocal_approx_fast(
        uv[:].rearrange("p g w -> p (g w)"), nrmb[:].rearrange("p g w -> p (g w)")
    )
    # e12n contiguous (strided-dst DVE writes run ~6x slower); scalar
    # mirrors it into Est rows 0:2 off-chain
    e12n = sb.tile([P, G, 2, 3], F32, tag=f"e12n{t}")
    nc.vector.tensor_mul(
        e12n[:], e12p[:], uv[:].unsqueeze(3).broadcast_to((P, G, 2, 3))
    )
    Est = sb.tile([P, G, 3, 3], F32, tag=f"Est{t}")
    nc.scalar.copy(Est[:, :, 0:2, :], e12n[:])
    # e3 = e1 x e2 via shifted duplicates (copies on scalar, off-chain)
    cbuf = sb.tile([P, G, 2, 6], F32, tag=f"cbuf{t}")
    nc.scalar.copy(cbuf[:, :, :, 0:3], e12n[:])
    nc.scalar.copy(cbuf[:, :, :, 3:6], e12n[:])
    mtmp = sb.tile([P, G, 2, 3], F32, tag=f"mtmp{t}")
    nc.vector.tensor_mul(mtmp[:, :, 0, :], cbuf[:, :, 0, 1:4], cbuf[:, :, 1, 2:5])
    nc.vector.tensor_mul(mtmp[:, :, 1, :], cbuf[:, :, 0, 2:5], cbuf[:, :, 1, 1:4])
    nc.vector.tensor_sub(Est[:, :, 2, :], mtmp[:, :, 0, :], mtmp[:, :, 1, :])

    # S = -2 Ep^T Et into Zb[8:17]
    Ev = Est[:].rearrange("p (c s) k x -> p c s k x", s=2)
    Ep = Ev[:, :, 0]  # [P, 8, 3(k), 3(x)]
    Et_ = Ev[:, :, 1]
    prodS = sb.tile([P, HALF, 9, 3], F32, tag=f"prodS{t}")
    for a in range(3):
        nc.vector.tensor_mul(
            prodS[:, :, 3 * a : 3 * a + 3, :],
            Ep[:, :, :, a].unsqueeze(2).broadcast_to((P, HALF, 3, 3)),
            Et_.transpose([0, 1, 3, 2]),
        )
    Rb = sb.tile([P, HALF, 9], F32, tag=f"Rb{t}")
    nc.vector.reduce_sum(Rb[:].unsqueeze(3), prodS[:], axis=mybir.AxisListType.X)
    nc.vector.tensor_scalar_mul(Zb[:, :, 8:17], Rb[:], -2.0)
    Sv = Zb[:, :, 8:17].rearrange("p c (a b) -> p c a b", a=3)

    # V = S bt, W = S^T bp
    prodv = sb.tile([P, HALF, 6, 3], F32, tag=f"prodv{t}")
    nc.vector.tensor_mul(
        prodv[:, :, 0:3, :], Sv, bt.unsqueeze(2).broadcast_to((P, HALF, 3, 3))
    )
    nc.vector.tensor_mul(
        prodv[:, :, 3:6, :],
        Sv.transpose([0, 1, 3, 2]),
        bp.unsqueeze(2).broadcast_to((P, HALF, 3, 3)),
    )
    VW = sb.tile([P, HALF, 2, 3], F32, tag=f"VW{t}")
    nc.vector.reduce_sum(
        VW[:].rearrange("p c v x -> p c (v x)").unsqueeze(3), prodv[:], axis=mybir.AxisListType.X
    )
    nc.vector.tensor_sub(
        Zb[:, :, 2:8].rearrange("p c (s x) -> p c s x", s=2), bpt_m2[:], VW[:]
    )
    # z0 = bp.(V + bp) + |bt|^2
    u1 = sb.tile([P, HALF, 3], F32, tag=f"u1{t}")
    nc.vector.tensor_add(u1[:], VW[:, :, 0, :], bp)
    nc.vector.tensor_mul(m2[:, :, 0, :], u1[:], bp)
    nc.vector.reduce_sum(Zb[:, :, 0:1], m2[:].rearrange("p c s x -> p c (s x)"), axis=mybir.AxisListType.X)
    # mask fold
    nc.vector.tensor_mul(
        Zb[:, :, 0:NF],
        Zb[:, :, 0:NF],
        Mj[:, h * HALF : (h + 1) * HALF].unsqueeze(2).broadcast_to((P, HALF, NF)),
    )


def _kernel_body(ctx, tc, out_dram, frames_in, coords_in, maskj_in, maski_in):
    nc = tc.nc
    P = 128
    sb = ctx.enter_context(tc.tile_pool(name="sb", bufs=1))
    outp = ctx.enter_context(tc.tile_pool(name="outp", bufs=4))
    ptr = ctx.enter_context(tc.tile_pool(name="ptr", bufs=2, space="PSUM"))
    pso = ctx.enter_context(tc.tile_pool(name="pso", bufs=3, space="PSUM"))

    mm_dt = mybir.dt.float32r if USE_F32R else F32

    # ---- input DMAs, issued in parallel from different engines ------------
    # frames split per half so chain A starts as soon as its half lands
    HB = NJCH * 2 * 9 // 2
    Ft = sb.tile([P, NJCH, 2, 3, 3], F32, tag="Ft")  # [p, chunk, set, pt, xyz]
    Ftf = Ft[:].rearrange("p c s t x -> p (c s t x)")
    nc.sync.dma_start(out=Ftf[:, 0:HB], in_=frames_in[:, 0:HB])
    nc.scalar.dma_start(out=Ftf[:, HB : 2 * HB], in_=frames_in[:, HB : 2 * HB])
    Ct = sb.tile([P, NITILE, 2, 3], F32, tag="Ct")  # [p, c, set, xyz]
    nc.gpsimd.dma_start(out=Ct[:].rearrange("p c s x -> p (c s x)"), in_=coords_in[:])
    Mj = sb.tile([P, NJCH], F32, tag="Mj")
    nc.sync.dma_start(out=Mj[:], in_=maskj_in[:])
    Mi = sb.tile([P, NITILE], F32, tag="Mi")
    nc.gpsimd.dma_start(out=Mi[:], in_=maski_in[:])

    # ---- early infra: identity, constants, ACT table preloads -------------
    scr = sb.tile([P, 2], F32, tag="scr")
    nc.gpsimd.memset(scr[:, 0:1], 1.0)
    bias_t = sb.tile([P, 1], F32, tag="bias")
    nc.gpsimd.memset(bias_t[:], SQRT_BIAS)
    # touch Square and Sqrt tables while waiting for inputs (each table
    # load is ~1.3us of scalar time; keep them off the critical path)
    nc.scalar.square(scr[:, 1:2], scr[:, 0:1])
    nc.scalar.sqrt(scr[:, 1:2], scr[:, 0:1])
    ident = sb.tile([P, P], F32, tag="ident")
    make_identity(nc, ident[:])

    ZbA = sb.tile([P, HALF, FPAD], F32, tag="ZbA")
    ZbB = sb.tile([P, HALF, FPAD], F32, tag="ZbB")
    Yb = sb.tile([P, NITILE, FPAD], F32, tag="Yb")
    nc.gpsimd.memset(ZbA[:, :, 1:2], 1.0)
    nc.gpsimd.memset(ZbB[:, :, 1:2], 1.0)
    nc.gpsimd.memset(Yb[:, :, 0:1], 1.0)

    # ---- Y features (coords only; gpsimd + one vector reduce) -------------
    sqc = sb.tile([P, NITILE, 6], F32, tag="sqc")
    nc.scalar.square(sqc[:].rearrange("p c x -> p (c x)"), Ct[:].rearrange("p c s x -> p (c s x)"))
    nc.gpsimd.tensor_copy(Yb[:, :, 2:8], Ct[:].rearrange("p c s x -> p c (s x)"))
    nc.vector.reduce_sum(Yb[:, :, 1:2], sqc[:], axis=mybir.AxisListType.X)
    nc.gpsimd.tensor_mul(
        Yb[:, :, 8:17].rearrange("p c (a b) -> p c a b", a=3),
        Ct[:, :, 0, :].unsqueeze(3).broadcast_to((P, NITILE, 3, 3)),
        Ct[:, :, 1, :].unsqueeze(2).broadcast_to((P, NITILE, 3, 3)),
    )
    nc.gpsimd.tensor_mul(
        Yb[:, :, 0:NF],
        Yb[:, :, 0:NF],
        Mi[:].unsqueeze(2).broadcast_to((P, NITILE, NF)),
    )

    # Y transpose -> YTrep replicated at row groups 0/32/64/96
    YTrep = sb.tile([P, ISLICE], mm_dt, tag="YTrep")
    YTf = YTrep[:].bitcast(F32)
    ptY = ptr.tile([P, 512], F32, tag="tp")
    nc.tensor.transpose(ptY[0:P, 0:P], Yb[:].rearrange("p c f -> p (c f)"), ident[:])
    for c in range(NITILE):
        nc.scalar.copy(YTrep[0:NF, c * P : (c + 1) * P], ptY[c * FPAD : c * FPAD + NF, 0:P])
    for r in range(1, 4):
        # scalar ACT copy; gpsimd's f32->f32r CAST path is ~3x slower
        nc.scalar.copy(YTrep[32 * r : 32 * r + NF, :], YTf[0:NF, :])

    # ---- two half-pipelines ----------------------------------------------
    for h, Zb in ((0, ZbA), (1, ZbB)):
        _half_chain(nc, sb, Zb, Ft, Mj, h)

        for g in range(2):  # transpose groups of 4 chunks within the half
            ptz = ptr.tile([P, 512], F32, tag="tp")
            nc.tensor.transpose(
                ptz[0:P, 0:P],
                Zb[:, 4 * g : 4 * g + 4, :].rearrange("p c f -> p (c f)"),
                ident[:],
            )
            zt_g = sb.tile([P, P], mm_dt, tag=f"ZT{h}{g}")
            if g % 2 == 0:
                nc.vector.tensor_copy(zt_g[:], ptz[0:P, 0:P])
            else:
                nc.scalar.copy(zt_g[:], ptz[0:P, 0:P])

            for pair in range(2):  # 2 chunk-pairs per transpose group
                pm = pso.tile([P, 1024], F32, tag="mm")
                for k in range(2):
                    r = 2 * pair + k  # row group index within the group
                    nc.tensor.matmul(
                        pm[:, 512 * k : 512 * (k + 1)],
                        zt_g[32 * r : 32 * r + NF, :],
                        YTrep[32 * r : 32 * r + NF, :],
                        start=True,
                        stop=True,
                        tile_position=(32 * r, 0),
                    )
                ot = outp.tile([P, 2, 512], F32, tag="ot")
                nc.scalar.activation(
                    ot[:].rearrange("p t q -> p (t q)"), pm[:, :], AF.Sqrt, bias=bias_t[:]
                )
                c0 = h * HALF + 4 * g + 2 * pair  # first chunk of the pair
                eng_d = [nc.sync, nc.gpsimd][(c0 // 2) % 2]
                eng_d.dma_start(
                    out=out_dram[c0 * P : (c0 + 2) * P, :].rearrange(
                        "(t p) q -> p t q", t=2
                    ),
                    in_=ot[:],
                )


def _shard_inputs(pred_coords, true_coords, pred_frames, true_frames, mask):
    """Host-side reformat into per-core DMA-friendly layouts."""
    pc = np.asarray(pred_coords, np.float32)
    tc = np.asarray(true_coords, np.float32)
    pf = np.asarray(pred_frames, np.float32)
    tf = np.asarray(true_frames, np.float32)
    mk = np.asarray(mask).astype(np.float32)

    in_maps = []
    for core in range(NCORES):
        b = core // (NCORES // B)
        i0 = (core % (NCORES // B)) * ISLICE
        # frames [128, chunk, set, pt, xyz] ; input frames are [n, xyz, pt]
        fr = np.stack([pf[b], tf[b]], axis=1)  # [n, 2, 3xyz, 3pt]
        fr = fr.transpose(0, 1, 3, 2)  # [n, 2, pt, xyz]
        fr = fr.reshape(NJCH, 128, 2, 3, 3).transpose(1, 0, 2, 3, 4)
        frames = np.ascontiguousarray(fr.reshape(128, -1))
        # coords [128, chunk, set, xyz]
        co = np.stack([pc[b, i0 : i0 + ISLICE], tc[b, i0 : i0 + ISLICE]], axis=1)
        co = co.reshape(NITILE, 128, 2, 3).transpose(1, 0, 2, 3)
        coords = np.ascontiguousarray(co.reshape(128, -1))
        maskj = np.ascontiguousarray(mk[b].reshape(NJCH, 128).T)
        maski = np.ascontiguousarray(mk[b, i0 : i0 + ISLICE].reshape(NITILE, 128).T)
        in_maps.append(
            {"frames": frames, "coords": coords, "maskj": maskj, "maski": maski}
        )
    return in_maps


def kernel(pred_coords, true_coords, pred_frames, true_frames, mask, _res=[]):
    nc = _build()
    in_maps = _shard_inputs(pred_coords, true_coords, pred_frames, true_frames, mask)
    res = run_bass_kernel_spmd(nc, in_maps, list(range(NCORES)))
    _res.clear()
    _res.append(res)
    out = np.empty((B, N, N), np.float32)
    for core in range(NCORES):
        b = core // (NCORES // B)
        i0 = (core % (NCORES // B)) * ISLICE
        out[b, i0 : i0 + ISLICE, :] = res.results[core]["out"].T
    return out


if __name__ == "__main__":
    rng = np.random.default_rng(0)
    ins = {
        "pred_coords": rng.standard_normal((B, N, 3)).astype(np.float32),
        "true_coords": rng.standard_normal((B, N, 3)).astype(np.float32),
        "pred_frames": rng.standard_normal((B, N, 3, 3)).astype(np.float32),
        "true_frames": rng.standard_normal((B, N, 3, 3)).astype(np.float32),
        "mask": np.ones((B, N), bool),
    }
    out = kernel(**ins)
    print("out", out.shape, out.dtype, float(np.abs(out).max()))


# revision 21
# speedup vs baseline: 1.1736x; 1.1736x over previous
"""ComputeAlignmentError kernel for 8 TRN2 NeuronCores.

Math: for each batch, pairwise alignment error
    err[i,j] = || Ep_j (pc_i - bp_j) - Et_j (tc_i - bt_j) + eps ||_2
where Ep/Et are orthonormal frame bases built from pred/true frames and
bp/bt the frame origins.  The eps terms contribute O(1e-8) relative and
are dropped; since Ep/Et are rotations the error collapses to a rank-17
bilinear form  err^2[i,j] = Y[i] . Z[j]:
    Y[i] = [1, |pc|^2+|tc|^2, pc, tc, vec(pc tc^T)]          (17)
    Z[j] = [z0, 1, -2bp - S bt, -2bt - S^T bp, vec(S)]       (17)
    S_j  = -2 Ep_j^T Et_j,   z0 = bp.(S bt + bp) + |bt|^2
Mask folds in for free: Y *= mask_i, Z *= mask_j.

Each core handles one (batch, 512-row i-slice).  Output is computed
j-major.  The j range is processed in TWO half-pipelines of 8 chunks
each so the scalar-engine sqrt drain of half A overlaps the vector
feature chain of half B: per half, Z features for 8x128 j are built
on-chip ([128 j, 8 chunks, 32 feat]), PE-transposed in groups of 4
chunks (chunk c lands at PE row group 32*(c%4) -- no replication for
Z), then one f32r matmul [17,128]x[17,512] per chunk into PSUM pairs,
sqrt over [128,1024] (scalar ACT, fused +bias guard against f32r
rounding pushing err^2<0), and a 512KB DMA per pair into a j-major
[2048, 512] output the host transposes back.  Y ([17, 512]) is built
once on scalar/gpsimd, PE-transposed, and replicated to all 4 row
groups with engine copies.
"""

import os
import sys

import numpy as np

sys.path.insert(0, "/opt/trn_rl_repo")

from contextlib import ExitStack

import concourse.bacc as bacc
import concourse.bass as bass
import concourse.tile as tile
from concourse import mybir
from concourse.bass_utils import run_bass_kernel_spmd
from concourse.masks import make_identity

F32 = mybir.dt.float32
AF = mybir.ActivationFunctionType

B, N = 2, 2048
NCORES = 8
ISLICE = N * B // NCORES  # 512 rows of i per core
NITILE = ISLICE // 128  # 4 i-chunks per core
NJCH = N // 128  # 16 j-chunks
NF = 17  # feature count K
FPAD = 32  # feature slot padding (PE row-group / PSUM alignment)
HALF = NJCH // 2  # chunks per half-pipeline

USE_F32R = True  # single-pass PE matmul; guarded by SQRT_BIAS
SQRT_BIAS = 2e-2 if USE_F32R else 2e-4


def _build(nc_holder=[]):
    if nc_holder:
        return nc_holder[0]
    nc = bacc.Bacc(
        "TRN2",
        target_bir_lowering=False,
        debug=False,
        enable_asserts=True,
        num_devices=NCORES,
    )
    # frames: [128, chunk, set, pt, xyz] (chunk-major so each half is
    # contiguous); coords: [128, chunk, set, xyz]
    frames_in = nc.dram_tensor("frames", [128, NJCH * 2 * 9], F32, kind="ExternalInput").ap()
    coords_in = nc.dram_tensor("coords", [128, NITILE * 6], F32, kind="ExternalInput").ap()
    maskj_in = nc.dram_tensor("maskj", [128, NJCH], F32, kind="ExternalInput").ap()
    maski_in = nc.dram_tensor("maski", [128, NITILE], F32, kind="ExternalInput").ap()
    out_dram = nc.dram_tensor("out", [N, ISLICE], F32, kind="ExternalOutput").ap()

    with tile.TileContext(nc) as tc, ExitStack() as ctx:
        _kernel_body(ctx, tc, out_dram, frames_in, coords_in, maskj_in, maski_in)

    nc.compile()
    nc_holder.append(nc)
    return nc


def _half_chain(nc, sb, Zb, Ft, Mj, h):
    """Emit the Z-feature chain for half h (chunks h*HALF .. h*HALF+7).

    Ft is the [P, NJCH, 2(set), 3(pt), 3(xyz)] frames tile; Zb is this
    half's [P, HALF, FPAD] feature buffer.  Vector carries the critical
    chain; scalar does squares/sqrts/copies that feed it.
    """
    P = 128
    G = 2 * HALF  # 16 groups: (chunk, set) chunk-major
    t = f"h{h}"
    Fh = Ft[:, h * HALF : (h + 1) * HALF]  # [P, 8, 2, 3, 3]
    Fg = Fh.rearrange("p c s t x -> p (c s) t x")  # [P, G, 3, 3]
    bp = Fh[:, :, 0, 1, :]  # [P, 8, 3]
    bt = Fh[:, :, 1, 1, :]

    # bpt_m2 = -2 * origins (gpsimd, off-chain)
    bpt_m2 = sb.tile([P, HALF, 2, 3], F32, tag=f"bptm2{t}")
    nc.gpsimd.tensor_scalar_mul(bpt_m2[:], Fh[:, :, :, 1, :], -2.0)
    # |bt|^2 terms for z0 (scalar, off-chain)
    m2 = sb.tile([P, HALF, 2, 3], F32, tag=f"m2{t}")
    nc.scalar.square(m2[:, :, 1, :], bt)

    w12 = sb.tile([P, G, 2, 3], F32, tag=f"w12{t}")
    nc.vector.tensor_sub(
        w12[:],
        Fg[:, :, 0::2, :],
        Fg[:, :, 1, :].unsqueeze(2).broadcast_to((P, G, 2, 3)),
    )
    pr = sb.tile([P, G, 3, 3], F32, tag=f"pr{t}")
    nc.scalar.square(pr[:, :, 0:2, :], w12[:])
    nc.vector.tensor_mul(pr[:, :, 2, :], w12[:, :, 0, :], w12[:, :, 1, :])
    dots = sb.tile([P, G, 3], F32, tag=f"dots{t}")
    nc.vector.reduce_sum(dots[:].unsqueeze(3), pr[:], axis=mybir.AxisListType.X)
    nrm12 = sb.tile([P, G, 2], F32, tag=f"nrm12{t}")
    nc.scalar.sqrt(nrm12[:], dots[:, :, 0:2])
    rinv12 = sb.tile([P, G, 2], F32, tag=f"rinv12{t}")
    nc.vector.reciprocal_approx_fast(
        rinv12[:].rearrange("p g w -> p (g w)"), nrm12[:].rearrange("p g w -> p (g w)")
    )
    w12n = sb.tile([P, G, 2, 3], F32, tag=f"w12n{t}")
    nc.vector.tensor_mul(w12n[:], w12[:], rinv12[:].unsqueeze(3).broadcast_to((P, G, 2, 3)))
    e12p = sb.tile([P, G, 2, 3], F32, tag=f"e12p{t}")
    nc.vector.tensor_add(e12p[:, :, 0, :], w12n[:, :, 0, :], w12n[:, :, 1, :])
    nc.vector.tensor_sub(e12p[:, :, 1, :], w12n[:, :, 1, :], w12n[:, :, 0, :])
    sq2 = sb.tile([P, G, 2, 3], F32, tag=f"sq2{t}")
    nc.scalar.square(sq2[:], e12p[:])
    n2b = sb.tile([P, G, 2], F32, tag=f"n2b{t}")
    nc.vector.reduce_sum(n2b[:].unsqueeze(3), sq2[:], axis=mybir.AxisListType.X)
    nrmb = sb.tile([P, G, 2], F32, tag=f"nrmb{t}")
    nc.scalar.sqrt(nrmb[:], n2b[:])
    uv = sb.tile([P, G, 2], F32, tag=f"uv{t}")
    nc.vector.reciprocal_approx_fast(
        uv[:].rearrange("p g w -> p (g w)"), nrmb[:].rearrange("p g w -> p (g w)")
    )
    # e12n contiguous (strided-dst DVE writes run ~6x slower); scalar
    # mirrors it into Est rows 0:2 off-chain
    e12n = sb.tile([P, G, 2, 3], F32, tag=f"e12n{t}")
    nc.vector.tensor_mul(
        e12n[:], e12p[:], uv[:].unsqueeze(3).broadcast_to((P, G, 2, 3))
    )
    Est = sb.tile([P, G, 3, 3], F32, tag=f"Est{t}")
    nc.scalar.copy(Est[:, :, 0:2, :], e12n[:])
    # e3 = e1 x e2 via shifted duplicates (copies on scalar, off-chain)
    cbuf = sb.tile([P, G, 2, 6], F32, tag=f"cbuf{t}")
    nc.scalar.copy(cbuf[:, :, :, 0:3], e12n[:])
    nc.scalar.copy(cbuf[:, :, :, 3:6], e12n[:])
    mtmp = sb.tile([P, G, 2, 3], F32, tag=f"mtmp{t}")
    nc.vector.tensor_mul(mtmp[:, :, 0, :], cbuf[:, :, 0, 1:4], cbuf[:, :, 1, 2:5])
    nc.vector.tensor_mul(mtmp[:, :, 1, :], cbuf[:, :, 0, 2:5], cbuf[:, :, 1, 1:4])
    nc.vector.tensor_sub(Est[:, :, 2, :], mtmp[:, :, 0, :], mtmp[:, :, 1, :])

    # S = -2 Ep^T Et into Zb[8:17]
    Ev = Est[:].rearrange("p (c s) k x -> p c s k x", s=2)
    Ep = Ev[:, :, 0]  # [P, 8, 3(k), 3(x)]
    Et_ = Ev[:, :, 1]
    prodS = sb.tile([P, HALF, 9, 3], F32, tag=f"prodS{t}")
    for a in range(3):
        nc.vector.tensor_mul(
            prodS[:, :, 3 * a : 3 * a + 3, :],
            Ep[:, :, :, a].unsqueeze(2).broadcast_to((P, HALF, 3, 3)),
            Et_.transpose([0, 1, 3, 2]),
        )
    Rb = sb.tile([P, HALF, 9], F32, tag=f"Rb{t}")
    nc.vector.reduce_sum(Rb[:].unsqueeze(3), prodS[:], axis=mybir.AxisListType.X)
    nc.vector.tensor_scalar_mul(Zb[:, :, 8:17], Rb[:], -2.0)
    Sv = Zb[:, :, 8:17].rearrange("p c (a b) -> p c a b", a=3)

    # V = S bt, W = S^T bp
    prodv = sb.tile([P, HALF, 6, 3], F32, tag=f"prodv{t}")
    nc.vector.tensor_mul(
        prodv[:, :, 0:3, :], Sv, bt.unsqueeze(2).broadcast_to((P, HALF, 3, 3))
    )
    nc.vector.tensor_mul(
        prodv[:, :, 3:6, :],
        Sv.transpose([0, 1, 3, 2]),
        bp.unsqueeze(2).broadcast_to((P, HALF, 3, 3)),
    )
    VW = sb.tile([P, HALF, 2, 3], F32, tag=f"VW{t}")
    nc.vector.reduce_sum(
        VW[:].rearrange("p c v x -> p c (v x)").unsqueeze(3), prodv[:], axis=mybir.AxisListType.X
    )
    nc.vector.tensor_sub(
        Zb[:, :, 2:8].rearrange("p c (s x) -> p c s x", s=2), bpt_m2[:], VW[:]
    )
    # z0 = bp.(V + bp) + |bt|^2
    u1 = sb.tile([P, HALF, 3], F32, tag=f"u1{t}")
    nc.vector.tensor_add(u1[:], VW[:, :, 0, :], bp)
    nc.vector.tensor_mul(m2[:, :, 0, :], u1[:], bp)
    nc.vector.reduce_sum(Zb[:, :, 0:1], m2[:].rearrange("p c s x -> p c (s x)"), axis=mybir.AxisListType.X)
    # mask fold
    nc.vector.tensor_mul(
        Zb[:, :, 0:NF],
        Zb[:, :, 0:NF],
        Mj[:, h * HALF : (h + 1) * HALF].unsqueeze(2).broadcast_to((P, HALF, NF)),
    )


def _kernel_body(ctx, tc, out_dram, frames_in, coords_in, maskj_in, maski_in):
    nc = tc.nc
    P = 128
    sb = ctx.enter_context(tc.tile_pool(name="sb", bufs=1))
    outp = ctx.enter_context(tc.tile_pool(name="outp", bufs=8))
    ptr = ctx.enter_context(tc.tile_pool(name="ptr", bufs=2, space="PSUM"))
    pso = ctx.enter_context(tc.tile_pool(name="pso", bufs=3, space="PSUM"))

    mm_dt = mybir.dt.float32r if USE_F32R else F32

    # ---- input DMAs: all on sync's queue (one warm ring; a per-engine
    # first-DMA pays ~2.5us ring latency), frames half A first ------------
    HB = NJCH * 2 * 9 // 2
    Ft = sb.tile([P, NJCH, 2, 3, 3], F32, tag="Ft")  # [p, chunk, set, pt, xyz]
    Ftf = Ft[:].rearrange("p c s t x -> p (c s t x)")
    nc.sync.dma_start(out=Ftf[:, 0:HB], in_=frames_in[:, 0:HB])
    Ct = sb.tile([P, NITILE, 2, 3], F32, tag="Ct")  # [p, c, set, xyz]
    nc.sync.dma_start(out=Ct[:].rearrange("p c s x -> p (c s x)"), in_=coords_in[:])
    nc.sync.dma_start(out=Ftf[:, HB : 2 * HB], in_=frames_in[:, HB : 2 * HB])
    Mj = sb.tile([P, NJCH], F32, tag="Mj")
    nc.sync.dma_start(out=Mj[:], in_=maskj_in[:])
    Mi = sb.tile([P, NITILE], F32, tag="Mi")
    nc.sync.dma_start(out=Mi[:], in_=maski_in[:])

    # ---- early infra: identity, constants, ACT table preloads -------------
    scr = sb.tile([P, 2], F32, tag="scr")
    nc.gpsimd.memset(scr[:, 0:1], 1.0)
    bias_t = sb.tile([P, 1], F32, tag="bias")
    nc.gpsimd.memset(bias_t[:], SQRT_BIAS)
    # touch Square and Sqrt tables while waiting for inputs (each table
    # load is ~1.3us of scalar time; keep them off the critical path)
    nc.scalar.square(scr[:, 1:2], scr[:, 0:1])
    nc.scalar.sqrt(scr[:, 1:2], scr[:, 0:1])
    ident = sb.tile([P, P], F32, tag="ident")
    make_identity(nc, ident[:])

    ZbA = sb.tile([P, HALF, FPAD], F32, tag="ZbA")
    ZbB = sb.tile([P, HALF, FPAD], F32, tag="ZbB")
    Yb = sb.tile([P, NITILE, FPAD], F32, tag="Yb")
    nc.gpsimd.memset(ZbA[:, :, 1:2], 1.0)
    nc.gpsimd.memset(ZbB[:, :, 1:2], 1.0)
    nc.gpsimd.memset(Yb[:, :, 0:1], 1.0)

    # ---- chain A first: its static-priority slots precede everything -----
    _half_chain(nc, sb, ZbA, Ft, Mj, 0)

    # ---- Y features (coords only; gpsimd + one vector reduce) -------------
    sqc = sb.tile([P, NITILE, 6], F32, tag="sqc")
    nc.scalar.square(sqc[:].rearrange("p c x -> p (c x)"), Ct[:].rearrange("p c s x -> p (c s x)"))
    nc.gpsimd.tensor_copy(Yb[:, :, 2:8], Ct[:].rearrange("p c s x -> p c (s x)"))
    nc.vector.reduce_sum(Yb[:, :, 1:2], sqc[:], axis=mybir.AxisListType.X)
    nc.gpsimd.tensor_mul(
        Yb[:, :, 8:17].rearrange("p c (a b) -> p c a b", a=3),
        Ct[:, :, 0, :].unsqueeze(3).broadcast_to((P, NITILE, 3, 3)),
        Ct[:, :, 1, :].unsqueeze(2).broadcast_to((P, NITILE, 3, 3)),
    )
    nc.gpsimd.tensor_mul(
        Yb[:, :, 0:NF],
        Yb[:, :, 0:NF],
        Mi[:].unsqueeze(2).broadcast_to((P, NITILE, NF)),
    )

    # Y transpose -> YTrep replicated at row groups 0/32/64/96
    YTrep = sb.tile([P, ISLICE], mm_dt, tag="YTrep")
    YTf = YTrep[:].bitcast(F32)
    ptY = ptr.tile([P, 512], F32, tag="tp")
    nc.tensor.transpose(ptY[0:P, 0:P], Yb[:].rearrange("p c f -> p (c f)"), ident[:])
    for c in range(NITILE):
        nc.scalar.copy(YTrep[0:NF, c * P : (c + 1) * P], ptY[c * FPAD : c * FPAD + NF, 0:P])
    for r in range(1, 4):
        # scalar ACT copy; gpsimd's f32->f32r CAST path is ~3x slower
        nc.scalar.copy(YTrep[32 * r : 32 * r + NF, :], YTf[0:NF, :])

    # ---- two half-pipelines (chain A already emitted above) ---------------
    for h, Zb in ((0, ZbA), (1, ZbB)):
        if h == 1:
            _half_chain(nc, sb, Zb, Ft, Mj, h)

        for g in range(2):  # transpose groups of 4 chunks within the half
            ptz = ptr.tile([P, 512], F32, tag="tp")
            nc.tensor.transpose(
                ptz[0:P, 0:P],
                Zb[:, 4 * g : 4 * g + 4, :].rearrange("p c f -> p (c f)"),
                ident[:],
            )
            zt_g = sb.tile([P, P], mm_dt, tag=f"ZT{h}{g}")
            if g % 2 == 0:
                nc.vector.tensor_copy(zt_g[:], ptz[0:P, 0:P])
            else:
                nc.scalar.copy(zt_g[:], ptz[0:P, 0:P])

            for pair in range(2):  # 2 chunk-pairs per transpose group
                pm = pso.tile([P, 1024], F32, tag="mm")
                for k in range(2):
                    r = 2 * pair + k  # row group index within the group
                    nc.tensor.matmul(
                        pm[:, 512 * k : 512 * (k + 1)],
                        zt_g[32 * r : 32 * r + NF, :],
                        YTrep[32 * r : 32 * r + NF, :],
                        start=True,
                        stop=True,
                        tile_position=(32 * r, 0),
                    )
                ot = outp.tile([P, 2, 512], F32, tag="ot")
                nc.scalar.activation(
                    ot[:].rearrange("p t q -> p (t q)"), pm[:, :], AF.Sqrt, bias=bias_t[:]
                )
                c0 = h * HALF + 4 * g + 2 * pair  # first chunk of the pair
                eng_d = [nc.sync, nc.gpsimd][(c0 // 2) % 2]
                eng_d.dma_start(
                    out=out_dram[c0 * P : (c0 + 2) * P, :].rearrange(
                        "(t p) q -> p t q", t=2
                    ),
                    in_=ot[:],
                )


def _shard_inputs(pred_coords, true_coords, pred_frames, true_frames, mask):
    """Host-side reformat into per-core DMA-friendly layouts."""
    pc = np.asarray(pred_coords, np.float32)
    tc = np.asarray(true_coords, np.float32)
    pf = np.asarray(pred_frames, np.float32)
    tf = np.asarray(true_frames, np.float32)
    mk = np.asarray(mask).astype(np.float32)

    in_maps = []
    for core in range(NCORES):
        b = core // (NCORES // B)
        i0 = (core % (NCORES // B)) * ISLICE
        # frames [128, chunk, set, pt, xyz] ; input frames are [n, xyz, pt]
        fr = np.stack([pf[b], tf[b]], axis=1)  # [n, 2, 3xyz, 3pt]
        fr = fr.transpose(0, 1, 3, 2)  # [n, 2, pt, xyz]
        fr = fr.reshape(NJCH, 128, 2, 3, 3).transpose(1, 0, 2, 3, 4)
        frames = np.ascontiguousarray(fr.reshape(128, -1))
        # coords [128, chunk, set, xyz]
        co = np.stack([pc[b, i0 : i0 + ISLICE], tc[b, i0 : i0 + ISLICE]], axis=1)
        co = co.reshape(NITILE, 128, 2, 3).transpose(1, 0, 2, 3)
        coords = np.ascontiguousarray(co.reshape(128, -1))
        maskj = np.ascontiguousarray(mk[b].reshape(NJCH, 128).T)
        maski = np.ascontiguousarray(mk[b, i0 : i0 + ISLICE].reshape(NITILE, 128).T)
        in_maps.append(
            {"frames": frames, "coords": coords, "maskj": maskj, "maski": maski}
        )
    return in_maps


def kernel(pred_coords, true_coords, pred_frames, true_frames, mask, _res=[]):
    nc = _build()
    in_maps = _shard_inputs(pred_coords, true_coords, pred_frames, true_frames, mask)
    res = run_bass_kernel_spmd(nc, in_maps, list(range(NCORES)))
    _res.clear()
    _res.append(res)
    out = np.empty((B, N, N), np.float32)
    for core in range(NCORES):
        b = core // (NCORES // B)
        i0 = (core % (NCORES // B)) * ISLICE
        out[b, i0 : i0 + ISLICE, :] = res.results[core]["out"].T
    return out


if __name__ == "__main__":
    rng = np.random.default_rng(0)
    ins = {
        "pred_coords": rng.standard_normal((B, N, 3)).astype(np.float32),
        "true_coords": rng.standard_normal((B, N, 3)).astype(np.float32),
        "pred_frames": rng.standard_normal((B, N, 3, 3)).astype(np.float32),
        "true_frames": rng.standard_normal((B, N, 3, 3)).astype(np.float32),
        "mask": np.ones((B, N), bool),
    }
    out = kernel(**ins)
    print("out", out.shape, out.dtype, float(np.abs(out).max()))
